# revision 3
# baseline (speedup 1.0000x reference)
"""Fixed-point (MPC) 3x3 VALID conv2d, NHWC, f32 — Trainium2 Bass kernel, v2.

Semantics (matches the jax reference to fp16-output rounding, fixed_point=8):
    qx = round_half_even(x*S)/S ; qw = round_half_even(w*S)/S
    y  = conv2d_valid(qx, qw)   ; out = floor(y*S)/S

Work split:
  HOST (not on the HW critical path):
    - quantize: x_int = rint(x*256) -> exact integers |.|<2048, stored fp16
    - layout:   per core, transpose to xb[(w,c)=3584, img=4, h=224]
    - weights:  banded lhsT wb[kh][16*dw+c, 16*j+k] = w_int[kh, dw-j, c, k]/256
                (w_int/256 is fp16-exact: same significand, exponent-8)
    - unpack:   y = yt_int16/256, reorder [blk,(j,k),(pair,ii),h'] -> NHWC
  DEVICE (8 cores, data-parallel over batch, 4 images/core):
    - per w-block b (37 blocks x 6 output w's = 222):
        DMA load xqt [128=(8w,16c), 4 img, 224 h] fp16   (1792B runs)
        per img-pair: 3 h-tap matmuls -> PSUM [96=(6w,16k), 2, 222] f32
          holding y_int2/256 EXACTLY (all values on the 1/256 grid, <2^24)
        one tensor_scalar: int16(psum - 255/512) == floor(y_int2/256)
          (f32->int16 convert is RNE on DVE and ACT; verified on HW)
        DMA store yt[b] [96, 4, 222] int16
  Engine budget/core: PE 98.6k cyc (conv only), DMA ~30MB eff, DVE/ACT ~20us.
"""

import numpy as np

import concourse.mybir as mybir
from concourse import bass, tile

N_CORES = 8
B_FULL = 32
B_CORE = B_FULL // N_CORES  # 4 images per core
H = W = 224
C = K = 16
HO = WO = 222

F32 = mybir.dt.float32
F16 = mybir.dt.float16
I16 = mybir.dt.int16

FLOOR_BIAS = -255.0 / 512.0  # int16(v + FLOOR_BIAS) == floor(v) for v on 1/256 grid
INV_S = 1.0 / 256.0

N_BLK = 37  # 37 blocks x 6 output w's = 222
WC = W * C  # 3584


def _split_multi_waits(nc):
    """The installed walrus only encodes ONE sync wait per instruction.
    Hoist extra waits onto NoOps inserted just before, same engine."""
    for f in nc.m.functions:
        for bb in f.blocks:
            new_list = []
            changed = False
            for ins in bb.instructions:
                si = ins.sync_info
                if si is not None and si.on_wait and len(si.on_wait) > 1:
                    waits = list(si.on_wait)
                    for wt in waits[:-1]:
                        nop = mybir.InstNoOp(
                            name=f"NOPW-{nc.next_id()}", ins=[], outs=[]
                        )
                        nop.engine = ins.engine
                        nop.sync_info = mybir.SyncInfo(on_wait=[wt], on_update=[])
                        new_list.append(nop)
                    ins.sync_info = mybir.SyncInfo(
                        on_wait=[waits[-1]], on_update=list(si.on_update or [])
                    )
                    changed = True
                new_list.append(ins)
            if changed:
                bb.instructions = new_list


def _build_nc(reps: int = 1):
    nc = bass.Bass("TRN2", num_devices=N_CORES)
    x_d = nc.dram_tensor("x", [WC, B_CORE, H], F16, kind="ExternalInput")
    wb_d = nc.dram_tensor("wb", [3, 128, 96], F16, kind="ExternalInput")
    y_d = nc.dram_tensor("y", [N_BLK, 96, B_CORE, WO], I16, kind="ExternalOutput")

    add = mybir.AluOpType.add
    COPY = mybir.ActivationFunctionType.Copy

    with tile.TileContext(nc) as tc:
        with (
            tc.tile_pool(name="consts", bufs=1) as consts,
            tc.tile_pool(name="xqt", bufs=8) as xqt_pool,
            tc.tile_pool(name="ub", bufs=6) as ub_pool,
            tc.tile_pool(name="psy", bufs=3, space="PSUM") as ps_pool,
            tc.tile_pool(name="warm", bufs=1, space="PSUM") as warm_pool,
        ):
            wtiles = []
            for kh in range(3):
                wt = consts.tile([128, 96], F16, tag=f"w{kh}")
                nc.sync.dma_start(out=wt[:], in_=wb_d[kh])
                wtiles.append(wt)

            # PE warmup: dependency-free matmuls on a zeroed tile ramp the
            # Tensor engine to full pstate while the first loads land.
            warm = consts.tile([128, 200], F16, tag="warm")
            nc.vector.memset(warm[:], 0.0)
            wps = warm_pool.tile([128, 200], F32, tag="wps")
            for i in range(16):
                nc.tensor.matmul(out=wps[:], lhsT=warm[:, :128], rhs=warm[:],
                                 start=True, stop=True)

            for rep in range(reps):
                for b in range(N_BLK):
                    xqt = xqt_pool.tile([128, B_CORE, H], F16, tag="xqt")
                    nc.sync.dma_start(
                        out=xqt[:], in_=x_d[96 * b : 96 * b + 128, :, :]
                    )
                    ub = ub_pool.tile([96, B_CORE, WO], I16, tag="ub")
                    for p in range(2):
                        psy = ps_pool.tile([96, 2, WO], F32, tag=f"psy{p}")
                        for s in range(3):
                            nc.tensor.matmul(
                                out=psy[:],
                                lhsT=wtiles[s][:],
                                rhs=xqt[:, 2 * p : 2 * p + 2, s : s + WO],
                                start=(s == 0),
                                stop=(s == 2),
                            )
                        if p == 0:
                            nc.vector.tensor_scalar(
                                out=ub[:, 0:2, :], in0=psy[:],
                                scalar1=FLOOR_BIAS, scalar2=None, op0=add,
                            )
                        else:
                            nc.scalar.activation(
                                out=ub[:, 2:4, :], in_=psy[:], func=COPY,
                                bias=FLOOR_BIAS, scale=1.0,
                            )
                    if b % 2 == 1:
                        nc.gpsimd.dma_start(out=y_d[b], in_=ub[:])
                    else:
                        nc.scalar.dma_start(out=y_d[b], in_=ub[:])

    _split_multi_waits(nc)
    return nc


def _banded_weights(w: np.ndarray) -> np.ndarray:
    """w [3,3,16,16] f32 -> wb [3, 128, 96] fp16 banded lhsT, scaled 1/256.

    wb[kh][16*dw + c, 16*j + k] = round(w*256)[kh, dw - j, c, k] / 256
    for 0 <= dw - j <= 2, j in 0..5."""
    wq = np.round(w.astype(np.float32) * np.float32(256.0))  # RNE, exact
    assert np.abs(wq).max() < 240, "w_int exceeds fp16-exact budget"
    wb = np.zeros((3, 128, 96), dtype=np.float32)
    for kh in range(3):
        for j in range(6):
            for kw in range(3):
                dw = j + kw
                wb[kh, 16 * dw : 16 * dw + 16, 16 * j : 16 * j + 16] = wq[kh, kw]
    return (wb * np.float32(INV_S)).astype(np.float16)


_RUNNER = None


def _get_runner():
    global _RUNNER
    if _RUNNER is None:
        _RUNNER = _make_runner(_build_nc())
    return _RUNNER


def _make_runner(nc):
    """Mirrors concourse.bass2jax.run_bass_via_pjrt's multi-core path but
    caches the jitted executable so repeated calls don't recompile."""
    import jax
    from jax.sharding import Mesh, PartitionSpec
    from jax.experimental.shard_map import shard_map
    from concourse.bass2jax import (
        _bass_exec_p,
        install_neuronx_cc_hook,
        partition_id_tensor,
    )

    install_neuronx_cc_hook()

    partition_name = nc.partition_id_tensor.name if nc.partition_id_tensor else None
    in_names, out_names, out_avals, zero_outs = [], [], [], []
    for alloc in nc.m.functions[0].allocations:
        if not isinstance(alloc, mybir.MemoryLocationSet):
            continue
        name = alloc.memorylocations[0].name
        if alloc.kind == "ExternalInput":
            if name != partition_name:
                in_names.append(name)
        elif alloc.kind == "ExternalOutput":
            out_names.append(name)
            shape = tuple(alloc.tensor_shape)
            dtype = mybir.dt.np(alloc.dtype)
            out_avals.append(jax.core.ShapedArray(shape, dtype))
            zero_outs.append(np.zeros(shape, dtype))
    n_params = len(in_names)
    all_in_names = list(in_names) + list(out_names)
    if partition_name is not None:
        all_in_names.append(partition_name)

    def _body(*args):
        operands = list(args)
        if partition_name is not None:
            operands.append(partition_id_tensor())
        outs = _bass_exec_p.bind(
            *operands,
            out_avals=tuple(out_avals),
            in_names=tuple(all_in_names),
            out_names=tuple(out_names),
            lowering_input_output_aliases=(),
            sim_require_finite=True,
            sim_require_nnan=True,
            nc=nc,
        )
        return tuple(outs)

    devices = jax.devices()[:N_CORES]
    assert len(devices) == N_CORES, f"need {N_CORES} devices, got {len(devices)}"
    mesh = Mesh(np.asarray(devices), ("core",))
    n_outs = len(out_avals)
    in_specs = (PartitionSpec("core"),) * (n_params + n_outs)
    out_specs = (PartitionSpec("core"),) * n_outs
    sharded = jax.jit(
        shard_map(_body, mesh=mesh, in_specs=in_specs, out_specs=out_specs,
                  check_rep=False),
        donate_argnums=tuple(range(n_params, n_params + n_outs)),
        keep_unused=True,
    )

    state = {
        "sharded": sharded,
        "in_names": in_names,
        "out_names": out_names,
        "out_avals": out_avals,
        "zero_outs": zero_outs,
        "n_cores": N_CORES,
    }

    def runner(in_maps):
        per_core = [[np.asarray(m[nm]) for nm in in_names] for m in in_maps]
        concat_in = [
            np.concatenate([per_core[c][i] for c in range(N_CORES)], axis=0)
            for i in range(n_params)
        ]
        concat_zeros = [
            np.zeros((N_CORES * z.shape[0], *z.shape[1:]), z.dtype)
            for z in zero_outs
        ]
        out_arrs = state["sharded"](*concat_in, *concat_zeros)
        return [
            {
                nm: np.asarray(out_arrs[i]).reshape(
                    N_CORES, *out_avals[i].shape
                )[c]
                for i, nm in enumerate(out_names)
            }
            for c in range(N_CORES)
        ]

    runner.state = state
    return runner


def _pack_inputs(x: np.ndarray) -> np.ndarray:
    """x [32, 224, 224, 16] f32 -> xb [32*224*16... ] fp16 in the per-core
    [(w,c), img, h] layout, concatenated over cores on the leading axis."""
    xi = np.rint(x.reshape(B_FULL, H, WC) * np.float32(256.0))  # RNE, exact
    xi16 = xi.astype(np.float16)  # |x_int| < 2048 -> exact
    # [core, img, h, wc] -> [core, wc, img, h]
    xb = np.ascontiguousarray(
        xi16.reshape(N_CORES, B_CORE, H, WC).transpose(0, 3, 1, 2)
    )
    return xb.reshape(N_CORES * WC, B_CORE, H)


def _unpack_output(yt: np.ndarray) -> np.ndarray:
    """yt [8 cores, 37, 96, 4, 222] int16 -> y [32, 222, 222, 16] f32."""
    # [core, blk, (j,k), (pair,ii), h'] -> [core, pair, ii, h', blk, j, k]
    yt = yt.reshape(N_CORES, N_BLK, 6, K, 2, 2, WO)
    yt = yt.transpose(0, 4, 5, 6, 1, 2, 3)  # int16 copy
    y = yt.astype(np.float32).reshape(B_FULL, HO, WO, K)
    y *= np.float32(INV_S)
    return y


def kernel(x: np.ndarray, w: np.ndarray, fixed_point) -> np.ndarray:
    assert int(fixed_point) == 8, f"kernel hardcodes fixed_point=8, got {fixed_point}"
    x = np.asarray(x, dtype=np.float32)
    assert x.shape == (B_FULL, H, W, C), x.shape
    assert np.abs(x).max() * 256.0 < 2040.0, "x_int exceeds fp16-exact budget"

    wb = _banded_weights(np.asarray(w, dtype=np.float32))
    xb = _pack_inputs(x)
    runner = _get_runner()

    in_maps = []
    for core in range(N_CORES):
        in_maps.append({"x": xb[WC * core : WC * (core + 1)], "wb": wb})

    results = runner(in_maps)
    yt = np.stack([r["y"] for r in results], axis=0)
    return _unpack_output(yt)


# revision 7
# speedup vs baseline: 1.0151x; 1.0151x over previous
"""Fixed-point (MPC) 3x3 VALID conv2d, NHWC, f32 — Trainium2 Bass kernel.

Semantics (matches the jax reference to fp16-output rounding, fixed_point=8):
    qx = round_half_even(x*S)/S ; qw = round_half_even(w*S)/S
    y  = conv2d_valid(qx, qw)   ; out = floor(y*S)/S

Work split:
  HOST (not on the HW critical path):
    - quantize: x_int = rint(x*256) -> exact integers |.|<2048, stored fp16
    - layout:   per core, transpose to xb[(w,c)=3584, img=4, h=224]
    - weights:  banded lhsT wb[kh][16*dw+c, 16*j+k] = w_int[kh, dw-j, c, k]/256
                (w_int/256 is fp16-exact: same significand, exponent-8)
    - unpack:   y = yt_int16/256, reorder [blk,(j,k),(pair,ii),h'] -> NHWC
  DEVICE (8 cores, data-parallel over batch, 4 images/core):
    - per w-block b (37 blocks x 6 output w's = 222):
        DMA load xqt [128=(8w,16c), 4 img, 224 h] fp16   (1792B runs)
        6 matmuls in tap-major order (each weight tile serves both image
        pairs back-to-back, alternating PSUM banks so consecutive matmuls
        never stall on the same bank's accumulation) -> two PSUM tiles
        [96=(6w,16k), 2, 222] f32 holding y_int2/256 EXACTLY (<2^24)
        one tensor_scalar: int16(psum - 255/512) == floor(y_int2/256)
          (f32->int16 convert is RNE on DVE and ACT; verified on HW)
        DMA store yt[b] [96, 4, 222] int16
  Engine budget/core: PE 98.6k cyc (conv only), DMA ~30MB eff, DVE/ACT ~20us.
"""

import numpy as np

import concourse.mybir as mybir
from concourse import bass, tile

N_CORES = 8
B_FULL = 32
B_CORE = B_FULL // N_CORES  # 4 images per core
H = W = 224
C = K = 16
HO = WO = 222

F32 = mybir.dt.float32
F16 = mybir.dt.float16
I16 = mybir.dt.int16

FLOOR_BIAS = -255.0 / 512.0  # int16(v + FLOOR_BIAS) == floor(v) for v on 1/256 grid
INV_S = 1.0 / 256.0

N_BLK = 37  # 37 blocks x 6 output w's = 222
WC = W * C  # 3584


def _split_multi_waits(nc):
    """The installed walrus only encodes ONE sync wait per instruction.
    Hoist extra waits onto NoOps inserted just before, same engine."""
    for f in nc.m.functions:
        for bb in f.blocks:
            new_list = []
            changed = False
            for ins in bb.instructions:
                si = ins.sync_info
                if si is not None and si.on_wait and len(si.on_wait) > 1:
                    waits = list(si.on_wait)
                    for wt in waits[:-1]:
                        nop = mybir.InstNoOp(
                            name=f"NOPW-{nc.next_id()}", ins=[], outs=[]
                        )
                        nop.engine = ins.engine
                        nop.sync_info = mybir.SyncInfo(on_wait=[wt], on_update=[])
                        new_list.append(nop)
                    ins.sync_info = mybir.SyncInfo(
                        on_wait=[waits[-1]], on_update=list(si.on_update or [])
                    )
                    changed = True
                new_list.append(ins)
            if changed:
                bb.instructions = new_list


def _build_nc(reps: int = 1):
    nc = bass.Bass("TRN2", num_devices=N_CORES)
    x_d = nc.dram_tensor("x", [WC, B_CORE, H], F16, kind="ExternalInput")
    wb_d = nc.dram_tensor("wb", [3, 128, 96], F16, kind="ExternalInput")
    y_d = nc.dram_tensor("y", [N_BLK, 96, B_CORE, WO], I16, kind="ExternalOutput")

    add = mybir.AluOpType.add
    COPY = mybir.ActivationFunctionType.Copy

    with tile.TileContext(nc) as tc:
        with (
            tc.tile_pool(name="consts", bufs=1) as consts,
            tc.tile_pool(name="xqt", bufs=8) as xqt_pool,
            tc.tile_pool(name="ub", bufs=6) as ub_pool,
            tc.tile_pool(name="psy", bufs=3, space="PSUM") as ps_pool,
            tc.tile_pool(name="warm", bufs=1, space="PSUM") as warm_pool,
        ):
            # Weight loads go via ACT (HWDGE) behind a small delay op so the
            # first x-tile load owns SP's sequencer and the HWDGE generator
            # at t=0; the first matmul needs wb only at ~2.8us.
            dly = consts.tile([1, 16], F32, tag="dly")
            nc.scalar.activation(out=dly[:], in_=dly[:],
                                 func=mybir.ActivationFunctionType.Copy,
                                 bias=0.0, scale=0.0)
            wtiles = []
            for kh in range(3):
                wt = consts.tile([128, 96], F16, tag=f"w{kh}")
                if kh == 0:
                    nc.scalar.dma_start(out=wt[:], in_=wb_d[kh])
                else:
                    nc.gpsimd.dma_start(out=wt[:], in_=wb_d[kh])
                wtiles.append(wt)

            # PE warmup: dependency-free matmuls on a zeroed tile ramp the
            # Tensor engine to full pstate while the first loads land.
            warm = consts.tile([128, 200], F16, tag="warm")
            nc.vector.memset(warm[:], 0.0)
            wps = warm_pool.tile([128, 200], F32, tag="wps")
            for i in range(14):
                nc.tensor.matmul(out=wps[:], lhsT=warm[:, :128], rhs=warm[:],
                                 start=True, stop=True)

            for rep in range(reps):
                for b in range(N_BLK):
                    xqt = xqt_pool.tile([128, B_CORE, H], F16, tag="xqt")
                    nc.sync.dma_start(
                        out=xqt[:], in_=x_d[96 * b : 96 * b + 128, :, :]
                    )
                    ub = ub_pool.tile([96, B_CORE, WO], I16, tag="ub")
                    # tap-major order: each weight tile serves both image
                    # pairs back-to-back (consecutive identical stationary
                    # weights), accumulating into two PSUM banks in parallel.
                    psys = [ps_pool.tile([96, 2, WO], F32, tag=f"psy{p}",
                                         name=f"psy{p}") for p in range(2)]
                    for s in range(3):
                        for p in range(2):
                            nc.tensor.matmul(
                                out=psys[p][:],
                                lhsT=wtiles[s][:],
                                rhs=xqt[:, 2 * p : 2 * p + 2, s : s + WO],
                                start=(s == 0),
                                stop=(s == 2),
                            )
                    for p in range(2):
                        psy = psys[p]
                        on_dve = (p == 0) or (b == N_BLK - 2)
                        if on_dve:
                            nc.vector.tensor_scalar(
                                out=ub[:, 2 * p : 2 * p + 2, :], in0=psy[:],
                                scalar1=FLOOR_BIAS, scalar2=None, op0=add,
                            )
                        else:
                            nc.scalar.activation(
                                out=ub[:, 2 * p : 2 * p + 2, :], in_=psy[:],
                                func=COPY, bias=FLOOR_BIAS, scale=1.0,
                            )
                    if b == N_BLK - 1:
                        nc.sync.dma_start(out=y_d[b, :, 0:2, :], in_=ub[:, 0:2, :])
                        nc.scalar.dma_start(out=y_d[b, :, 2:4, :], in_=ub[:, 2:4, :])
                    elif b == N_BLK - 2:
                        nc.sync.dma_start(out=y_d[b], in_=ub[:])
                    elif b % 2 == 1:
                        nc.gpsimd.dma_start(out=y_d[b], in_=ub[:])
                    else:
                        nc.scalar.dma_start(out=y_d[b], in_=ub[:])

    _split_multi_waits(nc)
    return nc


def _banded_weights(w: np.ndarray) -> np.ndarray:
    """w [3,3,16,16] f32 -> wb [3, 128, 96] fp16 banded lhsT, scaled 1/256.

    wb[kh][16*dw + c, 16*j + k] = round(w*256)[kh, dw - j, c, k] / 256
    for 0 <= dw - j <= 2, j in 0..5."""
    wq = np.round(w.astype(np.float32) * np.float32(256.0))  # RNE, exact
    assert np.abs(wq).max() < 240, "w_int exceeds fp16-exact budget"
    wb = np.zeros((3, 128, 96), dtype=np.float32)
    for kh in range(3):
        for j in range(6):
            for kw in range(3):
                dw = j + kw
                wb[kh, 16 * dw : 16 * dw + 16, 16 * j : 16 * j + 16] = wq[kh, kw]
    return (wb * np.float32(INV_S)).astype(np.float16)


_RUNNER = None


def _get_runner():
    global _RUNNER
    if _RUNNER is None:
        _RUNNER = _make_runner(_build_nc())
    return _RUNNER


def _make_runner(nc):
    """Mirrors concourse.bass2jax.run_bass_via_pjrt's multi-core path but
    caches the jitted executable so repeated calls don't recompile."""
    import jax
    from jax.sharding import Mesh, PartitionSpec
    from jax.experimental.shard_map import shard_map
    from concourse.bass2jax import (
        _bass_exec_p,
        install_neuronx_cc_hook,
        partition_id_tensor,
    )

    install_neuronx_cc_hook()

    partition_name = nc.partition_id_tensor.name if nc.partition_id_tensor else None
    in_names, out_names, out_avals, zero_outs = [], [], [], []
    for alloc in nc.m.functions[0].allocations:
        if not isinstance(alloc, mybir.MemoryLocationSet):
            continue
        name = alloc.memorylocations[0].name
        if alloc.kind == "ExternalInput":
            if name != partition_name:
                in_names.append(name)
        elif alloc.kind == "ExternalOutput":
            out_names.append(name)
            shape = tuple(alloc.tensor_shape)
            dtype = mybir.dt.np(alloc.dtype)
            out_avals.append(jax.core.ShapedArray(shape, dtype))
            zero_outs.append(np.zeros(shape, dtype))
    n_params = len(in_names)
    all_in_names = list(in_names) + list(out_names)
    if partition_name is not None:
        all_in_names.append(partition_name)

    def _body(*args):
        operands = list(args)
        if partition_name is not None:
            operands.append(partition_id_tensor())
        outs = _bass_exec_p.bind(
            *operands,
            out_avals=tuple(out_avals),
            in_names=tuple(all_in_names),
            out_names=tuple(out_names),
            lowering_input_output_aliases=(),
            sim_require_finite=True,
            sim_require_nnan=True,
            nc=nc,
        )
        return tuple(outs)

    devices = jax.devices()[:N_CORES]
    assert len(devices) == N_CORES, f"need {N_CORES} devices, got {len(devices)}"
    mesh = Mesh(np.asarray(devices), ("core",))
    n_outs = len(out_avals)
    in_specs = (PartitionSpec("core"),) * (n_params + n_outs)
    out_specs = (PartitionSpec("core"),) * n_outs
    sharded = jax.jit(
        shard_map(_body, mesh=mesh, in_specs=in_specs, out_specs=out_specs,
                  check_rep=False),
        donate_argnums=tuple(range(n_params, n_params + n_outs)),
        keep_unused=True,
    )

    state = {
        "sharded": sharded,
        "in_names": in_names,
        "out_names": out_names,
        "out_avals": out_avals,
        "zero_outs": zero_outs,
        "n_cores": N_CORES,
    }

    def runner(in_maps):
        per_core = [[np.asarray(m[nm]) for nm in in_names] for m in in_maps]
        concat_in = [
            np.concatenate([per_core[c][i] for c in range(N_CORES)], axis=0)
            for i in range(n_params)
        ]
        concat_zeros = [
            np.zeros((N_CORES * z.shape[0], *z.shape[1:]), z.dtype)
            for z in zero_outs
        ]
        out_arrs = state["sharded"](*concat_in, *concat_zeros)
        return [
            {
                nm: np.asarray(out_arrs[i]).reshape(
                    N_CORES, *out_avals[i].shape
                )[c]
                for i, nm in enumerate(out_names)
            }
            for c in range(N_CORES)
        ]

    runner.state = state
    return runner


def _pack_inputs(x: np.ndarray) -> np.ndarray:
    """x [32, 224, 224, 16] f32 -> xb [32*224*16... ] fp16 in the per-core
    [(w,c), img, h] layout, concatenated over cores on the leading axis."""
    xi = np.rint(x.reshape(B_FULL, H, WC) * np.float32(256.0))  # RNE, exact
    xi16 = xi.astype(np.float16)  # |x_int| < 2048 -> exact
    # [core, img, h, wc] -> [core, wc, img, h]
    xb = np.ascontiguousarray(
        xi16.reshape(N_CORES, B_CORE, H, WC).transpose(0, 3, 1, 2)
    )
    return xb.reshape(N_CORES * WC, B_CORE, H)


def _unpack_output(yt: np.ndarray) -> np.ndarray:
    """yt [8 cores, 37, 96, 4, 222] int16 -> y [32, 222, 222, 16] f32."""
    # [core, blk, (j,k), (pair,ii), h'] -> [core, pair, ii, h', blk, j, k]
    yt = yt.reshape(N_CORES, N_BLK, 6, K, 2, 2, WO)
    yt = yt.transpose(0, 4, 5, 6, 1, 2, 3)  # int16 copy
    y = yt.astype(np.float32).reshape(B_FULL, HO, WO, K)
    y *= np.float32(INV_S)
    return y


def kernel(x: np.ndarray, w: np.ndarray, fixed_point) -> np.ndarray:
    assert int(fixed_point) == 8, f"kernel hardcodes fixed_point=8, got {fixed_point}"
    x = np.asarray(x, dtype=np.float32)
    assert x.shape == (B_FULL, H, W, C), x.shape
    assert np.abs(x).max() * 256.0 < 2040.0, "x_int exceeds fp16-exact budget"

    wb = _banded_weights(np.asarray(w, dtype=np.float32))
    xb = _pack_inputs(x)
    runner = _get_runner()

    in_maps = []
    for core in range(N_CORES):
        in_maps.append({"x": xb[WC * core : WC * (core + 1)], "wb": wb})

    results = runner(in_maps)
    yt = np.stack([r["y"] for r in results], axis=0)
    return _unpack_output(yt)
